# revision 60
# baseline (speedup 1.0000x reference)
"""Multi-head attention (B=2, L=2048, D=1024, H=16) on 8 TRN2 NeuronCores.

Sharding: batch x head-group. Core c handles batch c//4 and heads
4*(c%4) .. 4*(c%4)+3. Each core:
  - projects its q/k/v slices (transposed activations fed from host),
  - runs flash-style attention in the "S-transposed" layout
    (keys on partitions, queries on free dim) so no on-device transposes
    are ever needed,
  - computes a partial output projection against its Wo column slice.
Host sums the 4 partials per batch.

Activations and weights stream in bf16 (half the HBM traffic of fp32;
same 1 cycle/row PE rate); all accumulation stays fp32 in PSUM, and the
exp/softmax pipeline runs on fp32 scores, so the output keeps <1e-2 rel
error. Partials leave the device in fp16 (host sums in fp64). Softmax
uses exp without max-subtraction (scores are O(1) by construction); the
attention mask folds into the exp bias, and the softmax denominator
comes for free from a ones-row appended to V.

Emission is software-pipelined: group-0 Q streams in 256-token halves so
the PE starts ~3us in; later projection groups interleave their V tiles
and Q/K half-chunks with the attention j-stream; output-projection
half-blocks fill the exp-bound steady-state groups; the last chunk's
output projection is split by head-pair plane, with the second plane's
partial summed on the host, so the drain stays short.

Engine notes (hardware-verified): Pool/gpsimd cannot read PSUM, so all
PSUM evacuation is on DVE and Pool only does partition broadcasts and
memsets; every deferred PV matmul must be EMITTED after the V tile it
reads (the tile framework cannot express a dep on a later-emitted
producer — such a race passes CoreSim by timing luck and NaNs on HW).
"""
import sys

sys.path.insert(0, "/opt/trn_rl_repo")

import numpy as np
import ml_dtypes
from contextlib import ExitStack, nullcontext

import concourse.bass as bass
import concourse.mybir as mybir
import concourse.tile as tile
from concourse import bacc
from concourse.bass import ts
from concourse.bass_utils import run_bass_kernel_spmd

F32 = mybir.dt.float32
F16 = mybir.dt.float16
BF16 = mybir.dt.bfloat16
EXP = mybir.ActivationFunctionType.Exp

B = 2
L = 2048
D = 1024
H = 16
DH = 64
HG = 4          # heads per core
NC = 8          # cores
P = 128
DT = D // P     # 8 d-tiles
JT = L // P     # 16 key tiles
IC = L // 512   # 4 query chunks of 512
G = 4           # projection token groups (512 tokens each)

_BUILT = None


def _build(loop_n=None):
    # loop_n: wrap the whole program in a hardware For_i loop executing it
    # loop_n times — used only by looptime.py for delta-wall calibration.
    nc = bacc.Bacc("TRN2", target_bir_lowering=False, debug=False, num_devices=1)

    xqT_d = nc.dram_tensor("xqT", (D, L), BF16, kind="ExternalInput").ap()
    xkT_d = nc.dram_tensor("xkT", (D, L), BF16, kind="ExternalInput").ap()
    xvT_d = nc.dram_tensor("xvT", (D, L), BF16, kind="ExternalInput").ap()
    wqT_d = nc.dram_tensor("wqT", (D, HG * DH), BF16, kind="ExternalInput").ap()
    wkT_d = nc.dram_tensor("wkT", (D, HG * DH), BF16, kind="ExternalInput").ap()
    wvT_d = nc.dram_tensor("wvT", (D, HG * DH), BF16, kind="ExternalInput").ap()
    woT_d = nc.dram_tensor("woT", (HG * DH, D), BF16, kind="ExternalInput").ap()
    mb_d = nc.dram_tensor("mb", (P, JT), F32, kind="ExternalInput").ap()
    # rows L..L+512 hold the pair-0 halves of the LAST query chunk's output
    # projection; the host adds them to rows 1536..2048 (saves the on-device
    # adds in the drain)
    out_d = nc.dram_tensor("partial", (L + 512, D), F16, kind="ExternalOutput").ap()

    xqT_r = xqT_d.rearrange("(dt p) t -> p dt t", p=P)
    xkT_r = xkT_d.rearrange("(dt p) t -> p dt t", p=P)
    xvT_r = xvT_d.rearrange("(dt p) t -> p dt t", p=P)

    marks = []

    def mark(label):
        marks.append((label, int(nc.get_next_instruction_name().split("-")[1])))

    with tile.TileContext(nc) as tc, ExitStack() as ctx:
        perm = ctx.enter_context(tc.tile_pool(name="perm", bufs=1))

        # resident weights
        wq = perm.tile([P, DT, HG * DH], BF16)
        wk = perm.tile([P, DT, HG * DH], BF16)
        wv = perm.tile([P, DT, HG * DH], BF16)
        wo = perm.tile([P, 2, D], BF16)
        mb = perm.tile([P, JT], F32)

        QT = [perm.tile([P, 2, 512], BF16, tag=f"QT{g}", name=f"QT{g}") for g in range(G)]
        KT = [perm.tile([P, 2, 512], BF16, tag=f"KT{g}", name=f"KT{g}") for g in range(G)]
        VT = [perm.tile([P, HG * (DH + 1)], BF16, tag=f"VT{j}", name=f"VT{j}") for j in range(JT)]
        OT = [perm.tile([P, 2, 512], BF16, tag=f"OT{g}", name=f"OT{g}") for g in range(G)]

        # group-0 activations: Q and K in 256-token halves (512B DMA
        # elements, full descriptor rate) so the PE starts ~3us in and the
        # first attention tiles chase the K halves; V whole
        xq0 = [perm.tile([P, DT, 256], BF16, name=f"xq0_{s}") for s in range(2)]
        xk0 = [perm.tile([P, DT, 256], BF16, name=f"xk0_{s}") for s in range(2)]
        xv0 = perm.tile([P, DT, 512], BF16, name="xv0")

        xpool = ctx.enter_context(tc.tile_pool(name="xg", bufs=6))
        spool = ctx.enter_context(tc.tile_pool(name="spool", bufs=2, space="PSUM"))
        ptpool = ctx.enter_context(tc.tile_pool(name="pt", bufs=7))
        stpool = ctx.enter_context(tc.tile_pool(name="st", bufs=4))
        small = ctx.enter_context(tc.tile_pool(name="small", bufs=2))

        def sslot():
            return spool.tile([P, 1024], F32, tag="s", name="s")

        # All PSUM->SBUF evacuations go through DVE: the Pool engine cannot
        # access PSUM on TRN2 (BIR verifier rejects it), so Pool only gets
        # SBUF-local work (partition broadcasts, memsets).
        def evac():
            return nc.vector

        # ---------------- projection emitters ----------------
        def proj_qk_slice(src, wt, dst, t0, nt):
            # out: [pair-heads on partitions, nt tokens] for both halves
            ps = sslot()
            for d in range(DT):
                for p in range(2):
                    nc.tensor.matmul(
                        ps[:, p * 512 + t0:p * 512 + t0 + nt], wt[:, d, ts(p, P)],
                        src[:, d, 0:nt],
                        start=(d == 0), stop=(d == DT - 1),
                    )
            for p in range(2):
                evac().tensor_copy(dst[:, p, t0:t0 + nt], ps[:, p * 512 + t0:p * 512 + t0 + nt])

        def proj_v_slice(src, jt, tloc):
            # V: natural layout [tokens, head dh]; the ones column for the
            # softmax denominator is memset once at startup
            psv = sslot()
            for d in range(DT):
                nc.tensor.matmul(
                    psv[:, 0:HG * DH],
                    src[:, d, ts(tloc, P)], wv[:, d, :],
                    start=(d == 0), stop=(d == DT - 1),
                )
            vg = VT[jt].rearrange("p (h c) -> p h c", c=DH + 1)
            evac().tensor_copy(
                vg[:, :, 0:DH],
                psv[:, 0:HG * DH].rearrange("p (h c) -> p h c", c=DH),
            )

        def proj_qk_half(src, wt, dst, p):
            # one p-half of a 512-token Q/K projection (~0.85us PE chunk)
            ps = sslot()
            for d in range(DT):
                nc.tensor.matmul(
                    ps[:, ts(p, 512)], wt[:, d, ts(p, P)], src[:, d, :],
                    start=(d == 0), stop=(d == DT - 1),
                )
            evac().tensor_copy(dst[:, p, :], ps[:, ts(p, 512)])

        # ---------------- attention ----------------
        # PV matmuls lag behind their exp in the PE stream so the PE never
        # stalls on the current j's exp; output-projection half-blocks are
        # sprinkled into later j-loops as PE filler.
        PENDING = []   # [(hp, pvs, j, pt)]
        FILLER = []    # deferred closures (oproj half-blocks)
        TAILF = []     # units reserved for group tails (cover exp latency)

        def flush_pv(keep):
            while len(PENDING) > keep:
                hp, pvs, j, pt = PENDING.pop(0)
                for h01 in range(2):
                    h = 2 * hp + h01
                    nc.tensor.matmul(
                        pvs[h01][:],
                        VT[j][:, h * (DH + 1):(h + 1) * (DH + 1)],
                        pt[:, ts(h01, 512)],
                        start=(j == 0), stop=(j == JT - 1),
                    )

        def attn_j(hp, ic, grp, j, keep):
            ps = sslot()
            nc.tensor.matmul(
                ps[:, 0:512],
                KT[j // 4][0:DH, hp, ts(j % 4, P)],
                QT[ic][0:DH, hp, :],
                start=True, stop=True,
            )
            nc.tensor.matmul(
                ps[:, 512:1024],
                KT[j // 4][DH:P, hp, ts(j % 4, P)],
                QT[ic][DH:P, hp, :],
                start=True, stop=True, tile_position=(DH, 0),
            )
            pt = ptpool.tile([P, 1024], BF16, tag="pt", name="pt")
            nc.scalar.activation(
                pt[:], ps[:], EXP, bias=mb[:, j:j + 1], scale=0.125,
            )
            PENDING.append((hp, grp["pvs"], j, pt))
            flush_pv(keep)

        LAG_KEEP = (4, 4, 4, 4, 3, 2)

        def attn_jseg(hp, ic, grp, j_range, fill=True, lag_head=False):
            for idx, j in enumerate(j_range):
                # deeper pv lag right after a group boundary: give the
                # previous group's norm chain cover before this group's
                # first PV needs the pvs banks back, then drain the backlog
                # one extra PV per j
                keep = LAG_KEEP[idx] if (lag_head and idx < len(LAG_KEEP)) else 1
                attn_j(hp, ic, grp, j, keep)
                if fill and FILLER and (idx == 0 or (idx >= 3 and idx % 2 == 1)):
                    FILLER.pop(0)()
            # a couple of tail units cover the last exp's latency before the
            # boundary flush
            if fill:
                for _ in range(2):
                    if TAILF:
                        TAILF.pop(0)()
                    elif FILLER:
                        FILLER.pop(0)()

        def attn_norm(hp, ic, grp):
            # normalize pv by the ones-row denominator, write OT.
            # chain per head: reciprocal (DVE) -> partition broadcast
            # (Pool) -> multiply (DVE; the pv-bank release path).
            recs, bcs = [], []
            for h01 in range(2):
                pv = grp["pvs"][h01]
                rec = small.tile([1, 512], F32, tag="rec", name="rec")
                nc.vector.reciprocal(rec[:], pv[DH:DH + 1, :])
                recs.append(rec)
            for h01 in range(2):
                bc = small.tile([DH, 512], F32, tag="bc", name="bc")
                nc.gpsimd.partition_broadcast(bc[:], recs[h01][:])
                bcs.append(bc)
            for h01 in range(2):
                nc.vector.tensor_mul(
                    out=OT[ic][ts(h01, DH), hp, :],
                    in0=grp["pvs"][h01][0:DH, :], in1=bcs[h01][:],
                )

        def finish_group_inline(hp, ic, grp):
            # flush this group's remaining PVs, then norm, emitted inline
            while any(pvs is grp["pvs"] for _, pvs, _, _ in PENDING):
                flush_pv(len(PENDING) - 1)
            attn_norm(hp, ic, grp)

        def new_grp(pvpool):
            return {
                "pvs": [
                    pvpool.tile([DH + 1, 512], F32, tag="pvA", name="pvA"),
                    pvpool.tile([DH + 1, 512], F32, tag="pvB", name="pvB"),
                ],
            }

        def make_oproj(auxpool):
            st_of = {}

            def get_st(tb, key=None):
                key = key if key is not None else tb
                if key not in st_of:
                    st_of[key] = stpool.tile([P, D], F16, tag="st", name="st")
                return st_of[key]

            def oproj_half(tb, mc):
                ic = tb // 4
                st = get_st(tb)
                pso = auxpool.tile([P, 512], F32, tag="pso", name="pso")
                for kt in range(2):
                    nc.tensor.matmul(
                        pso[:], OT[ic][:, kt, ts(tb % 4, P)],
                        wo[:, kt, ts(mc, 512)],
                        start=(kt == 0), stop=(kt == 1),
                    )
                evac().tensor_copy(st[:, ts(mc, 512)], pso[:])
                if mc == 1:
                    nc.sync.dma_start(out_d[ts(tb, P), :], st[:])
                    del st_of[tb]

            def oproj_kt0(tb, mc):
                # pair-0 plane of the LAST chunk's output projection, run as
                # filler inside the (1,3) j-loop and DMA'd to the extra
                # output rows (the host adds the two planes)
                ic = tb // 4
                st = get_st(tb, key=("kt0", tb))
                pso = auxpool.tile([P, 512], F32, tag="pso", name="pso")
                nc.tensor.matmul(
                    pso[:], OT[ic][:, 0, ts(tb % 4, P)],
                    wo[:, 0, ts(mc, 512)], start=True, stop=True,
                )
                evac().tensor_copy(st[:, ts(mc, 512)], pso[:])
                if mc == 1:
                    nc.sync.dma_start(out_d[ts(tb + 4, P), :], st[:])
                    del st_of[("kt0", tb)]

            def oproj_kt1(tb):
                # pair-1 plane after the final norm: both 512-halves into one
                # (idle) score-psum tile, f16 evac split DVE/ScalarE (the
                # exp stream is done, ScalarE is idle, and Copy lives in the
                # same act table as Exp so no table reload), one DMA
                ic = tb // 4
                pso = sslot()
                for mc in range(2):
                    nc.tensor.matmul(
                        pso[:, ts(mc, 512)], OT[ic][:, 1, ts(tb % 4, P)],
                        wo[:, 1, ts(mc, 512)], start=True, stop=True,
                    )
                st = get_st(tb)
                nc.vector.tensor_copy(st[:, 0:512], pso[:, 0:512])
                nc.scalar.activation(
                    st[:, 512:1024], pso[:, 512:1024],
                    mybir.ActivationFunctionType.Copy)
                nc.sync.dma_start(out_d[ts(tb, P), :], st[:])
                del st_of[tb]

            return oproj_half, oproj_kt0, oproj_kt1

        # ---------- pipelined emission ----------
        if loop_n:
            ctx.enter_context(tc.For_i(0, loop_n))
        # Prologue DMA, priority order: wq + all q slices (Q projections
        # start ~2.5us in), then wk + k slices (attention tiles chase the
        # K projections), then wv + v slices, then groups 1..3.
        # head: d-halved weight and first-activation DMAs so the first
        # projection sub-chunks (d 0-3) start as early as possible
        H2 = DT // 2
        wq_r = wqT_d.rearrange("(dt p) m -> p dt m", p=P)
        nc.sync.dma_start(wq[:, 0:H2, :], wq_r[:, 0:H2, :])
        nc.sync.dma_start(xq0[0][:, 0:H2, :], xqT_r[:, 0:H2, ts(0, 256)])
        nc.sync.dma_start(wq[:, H2:DT, :], wq_r[:, H2:DT, :])
        nc.sync.dma_start(xq0[0][:, H2:DT, :], xqT_r[:, H2:DT, ts(0, 256)])
        nc.sync.dma_start(xq0[1][:], xqT_r[:, :, ts(1, 256)])
        nc.sync.dma_start(mb[:], mb_d[:])
        wk_r = wkT_d.rearrange("(dt p) m -> p dt m", p=P)
        nc.sync.dma_start(wk[:, 0:H2, :], wk_r[:, 0:H2, :])
        nc.sync.dma_start(xk0[0][:, 0:H2, :], xkT_r[:, 0:H2, ts(0, 256)])
        nc.sync.dma_start(wk[:, H2:DT, :], wk_r[:, H2:DT, :])
        nc.sync.dma_start(xk0[0][:, H2:DT, :], xkT_r[:, H2:DT, ts(0, 256)])
        nc.sync.dma_start(xk0[1][:], xkT_r[:, :, ts(1, 256)])
        nc.sync.dma_start(wv[:], wvT_d.rearrange("(dt p) m -> p dt m", p=P))
        nc.sync.dma_start(xv0[:], xvT_r[:, :, 0:512])
        for j in range(JT):
            vg = VT[j].rearrange("p (h c) -> p h c", c=DH + 1)
            nc.gpsimd.memset(vg[:, :, DH:DH + 1], 1.0)
        mark("dma_pro")

        # Ramp: group-0 slice projections with the first attention tiles
        # chasing them; groups 1-3 stream behind the DMA with attention
        # groups (0,0) and (0,1) as PE filler. Dedicated 4-bank pv pool.
        with tc.tile_pool(name="rampv", bufs=2, space="PSUM") as rampv:
            g00 = new_grp(rampv)
            g01 = None
            for s in range(2):
                proj_qk_slice(xq0[s], wq, QT[0], 256 * s, 256)
            mark("projq0")
            # K projection in halves, the first QK/exp tiles chasing each
            # half; PVs stay pending (keep=4) so nothing waits on the V
            # projection yet
            for s in range(2):
                proj_qk_slice(xk0[s], wk, KT[0], 256 * s, 256)
                attn_j(0, 0, g00, 2 * s, keep=4)
                attn_j(0, 0, g00, 2 * s + 1, keep=4)
            mark("projk0")
            # V tiles, draining one pending PV after each
            for s in range(4):
                proj_v_slice(xv0, s, s)
                flush_pv(max(1, 3 - s))
            mark("g0")
            for g in range(1, G):
                xq_g = xpool.tile([P, DT, 512], BF16, tag="xg", name="xq_g")
                xk_g = xpool.tile([P, DT, 512], BF16, tag="xg", name="xk_g")
                xv_g = xpool.tile([P, DT, 512], BF16, tag="xg", name="xv_g")
                nc.sync.dma_start(xq_g[:], xqT_r[:, :, ts(g, 512)])
                nc.sync.dma_start(xk_g[:], xkT_r[:, :, ts(g, 512)])
                nc.sync.dma_start(xv_g[:], xvT_r[:, :, ts(g, 512)])
                # Q/K projections first (this group's attention needs K
                # complete), then V slices interleaved with the attention
                # j's so the PE never alternates between long proj-only and
                # exp-gated attention phases
                for p in range(2):
                    proj_qk_half(xq_g, wq, QT[g], p)
                for p in range(2):
                    proj_qk_half(xk_g, wk, KT[g], p)
                mark(f"proj{g}")
                if g01 is None:
                    g01 = new_grp(rampv)
                # Each slot s projects V tile 4g+s then runs `per` attention
                # j's. A PV for key tile j is EMITTED one attention-j after
                # its QK (keep=1), so (0,0)'s j must be the FIRST j of slot
                # j-4g: anything later would emit its PV before the V tile
                # it reads is projected (an untrackable race, not a stall).
                js = []
                if g < G - 1:
                    for i in range(4):
                        js.append((0, 0, g00, 4 * g + i))
                        js.append((0, 1, g01, 4 * (g - 1) + i))
                else:
                    rest = [(0, 1, g01, 8 + i) for i in range(8)]
                    for i in range(4):
                        js.append((0, 0, g00, 12 + i))
                        js.append(rest.pop(0))
                        js.append(rest.pop(0))
                per = len(js) // 4

                def emit_j():
                    hp, ic, grp, j = js.pop(0)
                    attn_j(hp, ic, grp, j, keep=1)
                    if (hp, ic, j) == (0, 0, JT - 1):
                        # (0,0) done: flush+norm now so it overlaps the rest
                        # of this group instead of the ramp tail
                        finish_group_inline(0, 0, g00)

                for slot in range(4):
                    proj_v_slice(xv_g, 4 * g + slot, slot)
                    for _ in range(per):
                        emit_j()
                while js:
                    emit_j()
                mark(f"attn_pipe{g}")
            finish_group_inline(0, 1, g01)
            mark("ramp_end")

        nc.sync.dma_start(wo[:], woT_d.rearrange("(kt p) m -> p kt m", p=P))

        # Steady state: interleave head-pairs so each query chunk's output
        # projection unlocks as early as possible and drains as filler.
        with tc.tile_pool(name="stpv", bufs=1, space="PSUM") as stpv, \
             tc.tile_pool(name="aux", bufs=2, space="PSUM") as auxpool:
            oproj_half, oproj_kt0, oproj_kt1 = make_oproj(auxpool)
            DRAINF = []

            def finish_group(hp, ic, grp):
                # deferred: popped at the NEXT group's first filler slot, so
                # this group's last PV (kept pending across the boundary)
                # flushes after its exp has long finished, and the norm runs
                # on DVE/Pool under the next group's QK stream.
                def _norm():
                    while any(pvs is grp["pvs"] for _, pvs, _, _ in PENDING):
                        flush_pv(len(PENDING) - 1)
                    attn_norm(hp, ic, grp)
                    if hp == 1 and ic < 3:
                        units = [lambda tb=tb, mc=mc: oproj_half(tb, mc)
                                 for tb in range(4 * ic, 4 * ic + 4)
                                 for mc in range(2)]
                        FILLER.extend(units[:6])
                        TAILF.extend(units[6:])
                    if hp == 0 and ic == 3:
                        # last chunk: the head-pair-0 half of its output
                        # projection can fill the (1,3) j-loop; the rest is
                        # reserved to cover the drain's final norm chain
                        units = [lambda tb=tb, mc=mc: oproj_kt0(tb, mc)
                                 for tb in range(12, 16)
                                 for mc in range(2)]
                        FILLER.extend(units[:4])
                        DRAINF.extend(units[4:])
                FILLER.insert(0, _norm)

            groups = [(1, 0), (1, 1), (0, 2), (1, 2), (0, 3), (1, 3)]
            for hp, ic in groups:
                grp = new_grp(stpv)
                attn_jseg(hp, ic, grp, range(JT), lag_head=True)
                finish_group(hp, ic, grp)
                mark(f"attn{hp}{ic}")
            flush_pv(0)
            while DRAINF:
                DRAINF.pop(0)()
            while TAILF:
                TAILF.pop(0)()
            while FILLER:
                FILLER.pop(0)()
            for tb in range(12, 16):
                oproj_kt1(tb)
            mark("drain")

    nc.compile()
    nc._phase_marks = marks
    return nc


def kernel(q, k, v, attention_mask, Wq, Wk, Wv, Wo):
    global _BUILT
    if _BUILT is None:
        _BUILT = _build()
    nc = _BUILT

    BF = ml_dtypes.bfloat16
    q = np.asarray(q, dtype=np.float32)
    k = np.asarray(k, dtype=np.float32)
    v = np.asarray(v, dtype=np.float32)
    Wq = np.asarray(Wq, dtype=np.float32)
    Wk = np.asarray(Wk, dtype=np.float32)
    Wv = np.asarray(Wv, dtype=np.float32)
    Wo = np.asarray(Wo, dtype=np.float32)
    mask = np.asarray(attention_mask)

    xT = {}
    for b in range(B):
        xT[("q", b)] = np.ascontiguousarray(q[b].T.astype(BF))
        xT[("k", b)] = np.ascontiguousarray(k[b].T.astype(BF))
        xT[("v", b)] = np.ascontiguousarray(v[b].T.astype(BF))

    in_maps = []
    for c in range(NC):
        b, hg = c // HG, c % HG
        rows = slice(hg * HG * DH, (hg + 1) * HG * DH)
        mbn = np.where(mask[b] == 0, np.float32(-1e9), np.float32(0.0))
        in_maps.append({
            "xqT": xT[("q", b)],
            "xkT": xT[("k", b)],
            "xvT": xT[("v", b)],
            "wqT": np.ascontiguousarray(Wq[rows].T.astype(BF)),
            "wkT": np.ascontiguousarray(Wk[rows].T.astype(BF)),
            "wvT": np.ascontiguousarray(Wv[rows].T.astype(BF)),
            "woT": np.ascontiguousarray(Wo[:, rows].T.astype(BF)),
            "mb": np.ascontiguousarray(mbn.reshape(JT, P).T),
        })

    res = run_bass_kernel_spmd(nc, in_maps, core_ids=list(range(NC)))
    kernel.last_results = res

    out = np.zeros((B, L, D), dtype=np.float64)
    for c in range(NC):
        part = res.results[c]["partial"].astype(np.float64)
        out[c // HG] += part[:L]
        # rows L.. hold the pair-0 plane of the last chunk's projection
        out[c // HG][L - 512:L] += part[L:]
    return out.astype(np.float32)
